# revision 9
# baseline (speedup 1.0000x reference)
"""Causal multi-head attention on 8 Trainium2 NeuronCores.

Problem: x[4,2048,1024], Wqkv[1024,3072] (H=16 heads, hd=64), causal mask,
softmax, Wproj[1024,1024] + bproj.

Sharding: (batch x head-group) across 8 cores. Core c handles batch b=c//2
and heads hg*8..hg*8+7 (hg=c%2). Each core computes QKV for its 512 head
columns, full causal attention for its 8 heads, and a partial output
projection over its 512 rows of Wproj. Host sums the two partials per batch
and adds the bias.

Device layouts (per core):
  xT   [1024, 2048]  x[b] transposed on host (d-major) - feeds all matmuls
  Q^T/K^T [512, 2048] in 4 sbuf tiles [128, 2048] (e-major)
  V    16 tiles [128(s), 512(e)]
  scores computed transposed: S^T[k,q] = K^T_blk.T @ Q^T  (k-blocks of 128,
  q-chunks of 512, causally skipped), exp on ScalarE (scale folded into Wq on
  host), P^T consumed by O^T = V_blk.T @ P^T and denom = ones.T @ P^T,
  normalize fused into the PSUM->SBUF copy (TT multiply with reciprocal).
  All matmuls run in float32r (TF32-like, 1 cyc/row at N>=512; ~1.5e-4 rel).
"""
import numpy as np

B, S, D, H = 4, 2048, 1024, 16
HD = D // H          # 64
HPC = H // 2         # 8 heads per core
SCALE = HD ** -0.5
NCORES = 8
SBK = S // 128       # 16 s-blocks
NQ = S // 512        # 4 q-chunks
KC = D // 128        # 8 d-chunks

_cache = {}


def _build():
    import concourse.mybir as mybir
    import concourse.tile as tile
    from concourse import bacc

    F32 = mybir.dt.float32
    F32R = mybir.dt.float32r
    Exp = mybir.ActivationFunctionType.Exp
    mult = mybir.AluOpType.mult

    nc = bacc.Bacc(None, target_bir_lowering=False)
    xT = nc.dram_tensor("xT", [D, S], F32, kind="ExternalInput")
    wq = nc.dram_tensor("wq", [D, 512], F32, kind="ExternalInput")
    wk = nc.dram_tensor("wk", [D, 512], F32, kind="ExternalInput")
    wv = nc.dram_tensor("wv", [D, 512], F32, kind="ExternalInput")
    wp = nc.dram_tensor("wp", [512, D], F32, kind="ExternalInput")
    tri = nc.dram_tensor("tri", [128, 128], F32, kind="ExternalInput")
    out = nc.dram_tensor("out", [S, D], F32, kind="ExternalOutput")

    with tile.TileContext(nc) as tc:
        with tc.tile_pool(name="pers", bufs=1) as pers, \
             tc.tile_pool(name="pab", bufs=1) as pab:
            tri_r = pers.tile([128, 128], F32R, name="tri_r")
            nc.gpsimd.dma_start(tri_r[:], tri[:])
            ones_f = pers.tile([128, 64], F32, name="ones_f")
            nc.vector.memset(ones_f[:], 1.0)
            ones_r = pers.tile([128, 64], F32R, name="ones_r")
            nc.vector.tensor_copy(ones_r[:], ones_f[:])

            qt = [pab.tile([128, S], F32R, name=f"qt{m}") for m in range(4)]
            kt = [pab.tile([128, S], F32R, name=f"kt{m}") for m in range(4)]
            vsb = [pab.tile([128, 512], F32R, name=f"v{i}") for i in range(SBK)]

            # ---------------- Phase A: QKV projection ----------------
            with tc.tile_pool(name="xtp", bufs=1) as xtp, \
                 tc.tile_pool(name="wpa", bufs=1) as wpa, \
                 tc.tile_pool(name="psA", bufs=4, space="PSUM") as psA:
                xt = [xtp.tile([128, S], F32R, name=f"xt{k}") for k in range(KC)]
                for k in range(KC):
                    nc.gpsimd.dma_start(xt[k][:], xT[k * 128:(k + 1) * 128, :])

                ncopy = 0

                def psum_out(dst, ps):
                    nonlocal ncopy
                    if ncopy % 2 == 0:
                        nc.vector.tensor_copy(dst, ps)
                    else:
                        nc.scalar.copy(dst, ps)
                    ncopy += 1

                for wdram, dst in ((wq, qt), (wk, kt)):
                    wt = [wpa.tile([128, 512], F32R, name=f"w{k}_{dst[0].tensor.name}",
                                   tag=f"w{k}") for k in range(KC)]
                    for k in range(KC):
                        nc.gpsimd.dma_start(wt[k][:], wdram[k * 128:(k + 1) * 128, :])
                    for m in range(4):
                        for n in range(NQ):
                            ps = psA.tile([128, 512], F32, name="psa", tag="psa")
                            for k in range(KC):
                                nc.tensor.matmul(
                                    ps[:], wt[k][:, m * 128:(m + 1) * 128],
                                    xt[k][:, n * 512:(n + 1) * 512],
                                    start=(k == 0), stop=(k == KC - 1))
                            psum_out(dst[m][:, n * 512:(n + 1) * 512], ps[:])
                wvt = [wpa.tile([128, 512], F32R, name=f"wv{k}", tag=f"w{k}")
                       for k in range(KC)]
                for k in range(KC):
                    nc.gpsimd.dma_start(wvt[k][:], wv[k * 128:(k + 1) * 128, :])
                for i in range(SBK):
                    ps = psA.tile([128, 512], F32, name="psa", tag="psa")
                    for k in range(KC):
                        nc.tensor.matmul(
                            ps[:], xt[k][:, i * 128:(i + 1) * 128], wvt[k][:],
                            start=(k == 0), stop=(k == KC - 1))
                    psum_out(vsb[i][:], ps[:])

            # per-head O^T rows (normalized), used in B and C; opened after
            # phase A so its SBUF doesn't overlap the xT tiles
            othp_cm = tc.tile_pool(name="othp", bufs=1, side="right")
            othp = othp_cm.__enter__()
            oth = [othp.tile([64, S], F32R, name=f"ot{h}") for h in range(HPC)]
            self_attention(nc, tc, mybir, qt, kt, vsb, oth, tri_r, ones_r)
        # pab (qt/kt/vsb) closed; phase C pools fit beside othp
        projection(nc, tc, mybir, oth, wp, out)
        othp_cm.__exit__(None, None, None)
    nc.finalize()
    return nc


def self_attention(nc, tc, mybir, qt, kt, vsb, oth, tri_r, ones_r):
    F32 = mybir.dt.float32
    F32R = mybir.dt.float32r
    Exp = mybir.ActivationFunctionType.Exp
    mult = mybir.AluOpType.mult
    if True:
            # ---------------- Phase B: causal attention ----------------
            with tc.tile_pool(name="ptp", bufs=2) as ptp, \
                 tc.tile_pool(name="rbp", bufs=2) as rbp, \
                 tc.tile_pool(name="psS", bufs=2, space="PSUM") as psS, \
                 tc.tile_pool(name="psO", bufs=2, space="PSUM") as psO:
                for h in range(HPC):
                    mt, pr = h // 2, (h % 2) * 64
                    for J in range(NQ):
                        nblk = 4 * J + 4
                        qs = slice(J * 512, (J + 1) * 512)
                        oav = psO.tile([64, 512], F32, name="oav", tag="oav")
                        odn = psO.tile([64, 512], F32, name="odn", tag="odn")
                        for g0 in range(0, nblk, 2):
                            grp = list(range(g0, min(g0 + 2, nblk)))
                            stg = psS.tile([128, 1024], F32, name="stg", tag="stg")
                            for gi, i in enumerate(grp):
                                nc.tensor.matmul(
                                    stg[:, gi * 512:(gi + 1) * 512],
                                    kt[mt][pr:pr + 64, i * 128:(i + 1) * 128],
                                    qt[mt][pr:pr + 64, qs],
                                    start=True, stop=True)
                            pt = ptp.tile([128, 1024], F32R, name="pt", tag="pt")
                            wg = len(grp) * 512
                            nc.scalar.activation(pt[:, :wg], stg[:, :wg], Exp)
                            for gi, i in enumerate(grp):
                                if i >= 4 * J:  # diagonal block: mask triangle
                                    w0 = 128 * i - 512 * J
                                    sl = pt[:, gi * 512 + w0: gi * 512 + w0 + 128]
                                    nc.vector.tensor_tensor(sl, sl, tri_r[:], op=mult)
                            for gi, i in enumerate(grp):
                                w0 = max(0, 128 * i - 512 * J)
                                psl = pt[:, gi * 512 + w0:(gi + 1) * 512]
                                nc.tensor.matmul(
                                    oav[:, w0:], vsb[i][:, h * 64:(h + 1) * 64], psl,
                                    start=(i == 0), stop=(i == nblk - 1))
                                nc.tensor.matmul(
                                    odn[:, w0:], ones_r[:], psl,
                                    start=(i == 0), stop=(i == nblk - 1))
                        rb = rbp.tile([64, 512], F32, name="rb", tag="rb")
                        nc.vector.reciprocal(rb[:], odn[:])
                        nc.vector.tensor_tensor(oth[h][:, qs], oav[:], rb[:], op=mult)


def projection(nc, tc, mybir, oth, wp, out):
    F32 = mybir.dt.float32
    F32R = mybir.dt.float32r
    with tc.tile_pool(name="wpc", bufs=1) as wpc, \
         tc.tile_pool(name="psC", bufs=2, space="PSUM") as psC, \
         tc.tile_pool(name="obp", bufs=3) as obp:
        wpt = [wpc.tile([64, D], F32R, name=f"wp{h}") for h in range(HPC)]
        for h in range(HPC):
            nc.gpsimd.dma_start(wpt[h][:], wp[h * 64:(h + 1) * 64, :])
        for s in range(SBK):
            pp = psC.tile([128, 1024], F32, name="pp", tag="pp")
            for n2 in range(2):
                for h in range(HPC):
                    nc.tensor.matmul(
                        pp[:, n2 * 512:(n2 + 1) * 512],
                        oth[h][:, s * 128:(s + 1) * 128],
                        wpt[h][:, n2 * 512:(n2 + 1) * 512],
                        start=(h == 0), stop=(h == HPC - 1))
            ob = obp.tile([128, 1024], F32, name="ob", tag="ob")
            nc.vector.tensor_copy(ob[:, 0:512], pp[:, 0:512])
            nc.scalar.copy(ob[:, 512:1024], pp[:, 512:1024])
            nc.sync.dma_start(out[s * 128:(s + 1) * 128, :], ob[:])


def _get_nc():
    if "nc" not in _cache:
        _cache["nc"] = _build()
    return _cache["nc"]


def kernel(x, mask, Wqkv, Wproj, bproj):
    from concourse.bass_utils import run_bass_kernel_spmd

    x = np.asarray(x, dtype=np.float32)
    Wqkv = np.asarray(Wqkv, dtype=np.float32)
    Wproj = np.asarray(Wproj, dtype=np.float32)
    bproj = np.asarray(bproj, dtype=np.float32)

    tri = np.ascontiguousarray(np.triu(np.ones((128, 128), dtype=np.float32)))
    xTs = [np.ascontiguousarray(x[b].T) for b in range(B)]
    in_maps = []
    for c in range(NCORES):
        b, hg = c // 2, c % 2
        cs = slice(hg * 512, (hg + 1) * 512)
        in_maps.append(dict(
            xT=xTs[b],
            wq=np.ascontiguousarray(Wqkv[:, 0 * D:1 * D][:, cs] * SCALE),
            wk=np.ascontiguousarray(Wqkv[:, 1 * D:2 * D][:, cs]),
            wv=np.ascontiguousarray(Wqkv[:, 2 * D:3 * D][:, cs]),
            wp=np.ascontiguousarray(Wproj[cs, :]),
            tri=tri,
        ))

    res = run_bass_kernel_spmd(_get_nc(), in_maps, core_ids=list(range(NCORES)),
                               **_cache.get("run_kwargs", {}))
    _cache["last_result"] = res
    parts = [r["out"] for r in res.results]
    outp = np.stack([parts[2 * b] + parts[2 * b + 1] for b in range(B)])
    outp += bproj
    return outp.astype(np.float32)


# revision 11
# speedup vs baseline: 1.2770x; 1.2770x over previous
"""Causal multi-head attention on 8 Trainium2 NeuronCores.

Problem: x[4,2048,1024], Wqkv[1024,3072] (H=16 heads, hd=64), causal mask,
softmax, Wproj[1024,1024] + bproj.

Sharding: (batch x head-group) across 8 cores. Core c handles batch b=c//2
and heads hg*8..hg*8+7 (hg=c%2). Each core computes QKV for its 512 head
columns, full causal attention for its 8 heads, and a partial output
projection over its 512 rows of Wproj. Host sums the two partials per batch
and adds the bias.

Device layouts (per core):
  xT   [1024, 2048]  x[b] transposed on host (d-major) - feeds all matmuls
  Q^T/K^T [512, 2048] in 4 sbuf tiles [128, 2048] (e-major)
  V    16 tiles [128(s), 512(e)]
  scores computed transposed: S^T[k,q] = K^T_blk.T @ Q^T  (k-blocks of 128,
  q-chunks of 512, causally skipped), exp on ScalarE (scale folded into Wq on
  host), P^T consumed by O^T = V_blk.T @ P^T and denom = ones.T @ P^T,
  normalize fused into the PSUM->SBUF copy (TT multiply with reciprocal).
  All matmuls run in float32r (TF32-like, 1 cyc/row at N>=512; ~1.5e-4 rel).
"""
import numpy as np

B, S, D, H = 4, 2048, 1024, 16
HD = D // H          # 64
HPC = H // 2         # 8 heads per core
SCALE = HD ** -0.5
NCORES = 8
SBK = S // 128       # 16 s-blocks
NQ = S // 512        # 4 q-chunks
KC = D // 128        # 8 d-chunks

_cache = {}


def _build():
    import concourse.mybir as mybir
    import concourse.tile as tile
    from concourse import bacc

    F32 = mybir.dt.float32
    F32R = mybir.dt.float32r
    Exp = mybir.ActivationFunctionType.Exp
    mult = mybir.AluOpType.mult

    nc = bacc.Bacc(None, target_bir_lowering=False)
    xT = nc.dram_tensor("xT", [D, S], F32, kind="ExternalInput")
    wq = nc.dram_tensor("wq", [D, 512], F32, kind="ExternalInput")
    wk = nc.dram_tensor("wk", [D, 512], F32, kind="ExternalInput")
    wv = nc.dram_tensor("wv", [D, 512], F32, kind="ExternalInput")
    wp = nc.dram_tensor("wp", [512, D], F32, kind="ExternalInput")
    tri = nc.dram_tensor("tri", [128, 128], F32, kind="ExternalInput")
    out = nc.dram_tensor("out", [S, D], F32, kind="ExternalOutput")

    with tile.TileContext(nc) as tc:
        with tc.tile_pool(name="pers", bufs=1) as pers, \
             tc.tile_pool(name="pab", bufs=1) as pab:
            tri_r = pers.tile([128, 128], F32R, name="tri_r")
            nc.gpsimd.dma_start(tri_r[:], tri[:])
            ones_f = pers.tile([128, 64], F32, name="ones_f")
            nc.vector.memset(ones_f[:], 1.0)
            ones_r = pers.tile([128, 64], F32R, name="ones_r")
            nc.vector.tensor_copy(ones_r[:], ones_f[:])

            qt = [pab.tile([128, S], F32R, name=f"qt{m}") for m in range(4)]
            kt = [pab.tile([128, S], F32R, name=f"kt{m}") for m in range(4)]
            vsb = [pab.tile([128, 512], F32R, name=f"v{i}") for i in range(SBK)]

            # ---------------- Phase A: QKV projection ----------------
            with tc.tile_pool(name="xtp", bufs=1) as xtp, \
                 tc.tile_pool(name="wpa", bufs=1) as wpa, \
                 tc.tile_pool(name="psA", bufs=4, space="PSUM") as psA:
                xt = [xtp.tile([128, S], F32R, name=f"xt{k}") for k in range(KC)]

                ncopy = 0

                def psum_out(dst, ps):
                    nonlocal ncopy
                    if ncopy % 2 == 0:
                        nc.vector.tensor_copy(dst, ps)
                    else:
                        nc.scalar.copy(dst, ps)
                    ncopy += 1

                for wdram, dst in ((wq, qt), (wk, kt)):
                    wt = [wpa.tile([128, 512], F32R, name=f"w{k}_{dst[0].tensor.name}",
                                   tag=f"w{k}") for k in range(KC)]
                    for k in range(KC):
                        nc.gpsimd.dma_start(wt[k][:], wdram[k * 128:(k + 1) * 128, :])
                        if wdram is wq:
                            nc.gpsimd.dma_start(xt[k][:],
                                                xT[k * 128:(k + 1) * 128, :])
                    for m in range(4):
                        for n in range(NQ):
                            ps = psA.tile([128, 512], F32, name="psa", tag="psa")
                            for k in range(KC):
                                nc.tensor.matmul(
                                    ps[:], wt[k][:, m * 128:(m + 1) * 128],
                                    xt[k][:, n * 512:(n + 1) * 512],
                                    start=(k == 0), stop=(k == KC - 1))
                            psum_out(dst[m][:, n * 512:(n + 1) * 512], ps[:])
                wvt = [wpa.tile([128, 512], F32R, name=f"wv{k}", tag=f"w{k}")
                       for k in range(KC)]
                for k in range(KC):
                    nc.gpsimd.dma_start(wvt[k][:], wv[k * 128:(k + 1) * 128, :])
                for i in range(SBK):
                    ps = psA.tile([128, 512], F32, name="psa", tag="psa")
                    for k in range(KC):
                        nc.tensor.matmul(
                            ps[:], xt[k][:, i * 128:(i + 1) * 128], wvt[k][:],
                            start=(k == 0), stop=(k == KC - 1))
                    psum_out(vsb[i][:], ps[:])

            # per-head O^T rows (normalized), used in B and C; opened after
            # phase A so its SBUF doesn't overlap the xT tiles
            othp_cm = tc.tile_pool(name="othp", bufs=1, side="right")
            othp = othp_cm.__enter__()
            oth = [othp.tile([64, S], F32R, name=f"ot{h}") for h in range(HPC)]
            wpc_cm = tc.tile_pool(name="wpc", bufs=1, side="right")
            wpcp = wpc_cm.__enter__()
            wpt = [wpcp.tile([64, D], F32R, name=f"wp{h}") for h in range(HPC)]
            for h in range(HPC):
                nc.gpsimd.dma_start(wpt[h][:], wp[h * 64:(h + 1) * 64, :])
            self_attention(nc, tc, mybir, qt, kt, vsb, oth, tri_r, ones_r)
        # pab (qt/kt/vsb) closed; phase C pools fit beside othp
        projection(nc, tc, mybir, oth, wpt, out)
        wpc_cm.__exit__(None, None, None)
        othp_cm.__exit__(None, None, None)
    nc.finalize()
    return nc


def self_attention(nc, tc, mybir, qt, kt, vsb, oth, tri_r, ones_r):
    F32 = mybir.dt.float32
    F32R = mybir.dt.float32r
    Exp = mybir.ActivationFunctionType.Exp
    mult = mybir.AluOpType.mult
    if True:
            # ---------------- Phase B: causal attention ----------------
            with tc.tile_pool(name="ptp", bufs=2) as ptp, \
                 tc.tile_pool(name="rbp", bufs=2) as rbp, \
                 tc.tile_pool(name="psS", bufs=2, space="PSUM") as psS, \
                 tc.tile_pool(name="psO", bufs=2, space="PSUM") as psO:
                for h in range(HPC):
                    mt, pr = h // 2, (h % 2) * 64
                    for J in range(NQ):
                        nblk = 4 * J + 4
                        qs = slice(J * 512, (J + 1) * 512)
                        oav = psO.tile([64, 512], F32, name="oav", tag="oav")
                        odn = psO.tile([64, 512], F32, name="odn", tag="odn")
                        for g0 in range(0, nblk, 2):
                            grp = list(range(g0, min(g0 + 2, nblk)))
                            stg = psS.tile([128, 1024], F32, name="stg", tag="stg")
                            for gi, i in enumerate(grp):
                                nc.tensor.matmul(
                                    stg[:, gi * 512:(gi + 1) * 512],
                                    kt[mt][pr:pr + 64, i * 128:(i + 1) * 128],
                                    qt[mt][pr:pr + 64, qs],
                                    start=True, stop=True)
                            pt = ptp.tile([128, 1024], F32R, name="pt", tag="pt")
                            wg = len(grp) * 512
                            nc.scalar.activation(pt[:, :wg], stg[:, :wg], Exp)
                            for gi, i in enumerate(grp):
                                if i >= 4 * J:  # diagonal block: mask triangle
                                    w0 = 128 * i - 512 * J
                                    sl = pt[:, gi * 512 + w0: gi * 512 + w0 + 128]
                                    nc.vector.tensor_tensor(sl, sl, tri_r[:], op=mult)
                            for gi, i in enumerate(grp):
                                w0 = max(0, 128 * i - 512 * J)
                                psl = pt[:, gi * 512 + w0:(gi + 1) * 512]
                                nc.tensor.matmul(
                                    oav[:, w0:], vsb[i][:, h * 64:(h + 1) * 64], psl,
                                    start=(i == 0), stop=(i == nblk - 1))
                                nc.tensor.matmul(
                                    odn[:, w0:], ones_r[:], psl,
                                    start=(i == 0), stop=(i == nblk - 1))
                        rb = rbp.tile([64, 512], F32, name="rb", tag="rb")
                        nc.vector.reciprocal_approx_fast(rb[:], odn[:])
                        nc.vector.tensor_tensor(oth[h][:, qs], oav[:], rb[:], op=mult)


def projection(nc, tc, mybir, oth, wpt, out):
    F32 = mybir.dt.float32
    with tc.tile_pool(name="psC", bufs=2, space="PSUM") as psC, \
         tc.tile_pool(name="obp", bufs=3) as obp:
        for s in range(SBK):
            pp = psC.tile([128, 1024], F32, name="pp", tag="pp")
            for n2 in range(2):
                for h in range(HPC):
                    nc.tensor.matmul(
                        pp[:, n2 * 512:(n2 + 1) * 512],
                        oth[h][:, s * 128:(s + 1) * 128],
                        wpt[h][:, n2 * 512:(n2 + 1) * 512],
                        start=(h == 0), stop=(h == HPC - 1))
            ob = obp.tile([128, 1024], F32, name="ob", tag="ob")
            nc.vector.tensor_copy(ob[:, 0:512], pp[:, 0:512])
            nc.scalar.copy(ob[:, 512:1024], pp[:, 512:1024])
            nc.sync.dma_start(out[s * 128:(s + 1) * 128, :], ob[:])


def _get_nc():
    if "nc" not in _cache:
        _cache["nc"] = _build()
    return _cache["nc"]


def kernel(x, mask, Wqkv, Wproj, bproj):
    from concourse.bass_utils import run_bass_kernel_spmd

    x = np.asarray(x, dtype=np.float32)
    Wqkv = np.asarray(Wqkv, dtype=np.float32)
    Wproj = np.asarray(Wproj, dtype=np.float32)
    bproj = np.asarray(bproj, dtype=np.float32)

    tri = np.ascontiguousarray(np.triu(np.ones((128, 128), dtype=np.float32)))
    xTs = [np.ascontiguousarray(x[b].T) for b in range(B)]
    in_maps = []
    for c in range(NCORES):
        b, hg = c // 2, c % 2
        cs = slice(hg * 512, (hg + 1) * 512)
        in_maps.append(dict(
            xT=xTs[b],
            wq=np.ascontiguousarray(Wqkv[:, 0 * D:1 * D][:, cs] * SCALE),
            wk=np.ascontiguousarray(Wqkv[:, 1 * D:2 * D][:, cs]),
            wv=np.ascontiguousarray(Wqkv[:, 2 * D:3 * D][:, cs]),
            wp=np.ascontiguousarray(Wproj[cs, :]),
            tri=tri,
        ))

    res = run_bass_kernel_spmd(_get_nc(), in_maps, core_ids=list(range(NCORES)),
                               **_cache.get("run_kwargs", {}))
    _cache["last_result"] = res
    parts = [r["out"] for r in res.results]
    outp = np.stack([parts[2 * b] + parts[2 * b + 1] for b in range(B)])
    outp += bproj
    return outp.astype(np.float32)
